# revision 24
# baseline (speedup 1.0000x reference)
"""AdvancedVectorMemory fused kernel for 8 Trainium2 NeuronCores.

Sharding: core c handles batch b = c//4 and heads 4*(c%4) .. 4*(c%4)+3
(data parallel over batch, tensor parallel over heads). Attention runs
flash-style per head pair with fused denominators (ones column in V).

Perf structure:
 - s-rotation: core (b, g) processes logical s-slices in the order
   g+1, g+2, g+3, g (mod 4), host-side permutation of q columns. Its
   own slice is computed LAST, so only 3 AllGathers are needed; each AG
   overlaps the next chunk pair's compute.
 - exp is split across TWO engines: even strips use ScalarE's real Exp;
   odd strips use a Schraudolph bit-trick on VectorE (logit*184.665 +
   magic constant in f32; the low 16 bits of the f32 sum ARE the bf16
   bit pattern of ~exp(logit), read back via bitcast + stride-2 AP).
   Softmax renormalization absorbs the +-3% systematic error.
 - rt (attention accumulator) copies run on ScalarE, freeing VectorE
   for the magic-exp strips.
 - Wo / Wg1 ship from host as bf16 (half the DMA, no on-chip casts);
   qs ships both f32 (residual) and bf16 (matmul).
 - Startup: wq + first q chunks DMA first; small consts go to the
   gpsimd queue; big preloads issue from the scalar queue mid-attention.
 - AllGather outputs are Shared-scratchpad DRAM (fast HBM-HBM path).
 - Gathers + denominator reciprocals run during attention; the epilogue
   normalizes early chunks first so Wo can start immediately.
"""
import sys
import numpy as np

for _p in ('/opt/trn_rl_repo', '/root/.axon_site/_ro/trn_rl_repo'):
    if _p not in sys.path:
        sys.path.insert(0, _p)

B, S, M = 2, 2048, 4096
DM, DK = 1024, 768
H, Dh = 16, 64
NC = 8
GS = 4           # group size (cores per batch)
SC_W = 512       # s-chunk width
N_SC = S // SC_W
N_MT = M // 128  # 32 m-tiles
SSL = S // GS    # per-core s-slice for the epilogue (512)

# Schraudolph fp8e4m3 exp-by-bits on VectorE: int8 convert of
# x*(8/ln2) + (56 - c); the int8 bits ARE the fp8 pattern of ~exp(x).
# The +-3% systematic error renormalizes out in the softmax.
SCH8_SCALE = 8.0 / float(np.log(2.0))
SCH8_MAGIC = 56.0 - 0.344


_PROG = None


def _build_program():
    from concourse import bacc, mybir, tile
    import concourse.bass as bass

    F32 = mybir.dt.float32
    F32R = mybir.dt.float32r
    BF16 = mybir.dt.bfloat16
    FP8 = mybir.dt.float8e4
    I8 = mybir.dt.int8
    AF = mybir.ActivationFunctionType
    ALU = mybir.AluOpType
    PM = mybir.MatmulPerfMode

    nc = bacc.Bacc('TRN2', target_bir_lowering=False, debug=False, num_devices=NC)

    def din(name, shape, dt=F32R):
        return nc.dram_tensor(name, shape, dt, kind='ExternalInput').ap()

    qT = din('qT', [DM, S])
    mkT = din('mkT', [DK, M])
    mvT = din('mvT', [DK, M])
    wqT = din('wqT', [DM, 256])
    wkT = din('wkT', [DK, 256])
    wvT = din('wvT', [DK, 256])
    woT = din('woT', [DM, DM], FP8)        # 16*Wo
    wg1T = din('wg1T', [2 * DM, DM], FP8)  # 16*Wg1
    wg2T = din('wg2T', [DM, 2])
    qsT = din('qsT', [DM, SSL], F32)
    qsbT = din('qsbT', [DM, SSL], FP8)     # qs/16
    bc0 = din('bc0', [2, 128])        # row0 = ones (gate broadcast)
    bqv = din('bqv', [2, 128], F32)
    bkv = din('bkv', [2, 128], F32)
    bo2v = din('bo2v', [8, 128], F32)
    bg1v = din('bg1v', [8, 128], F32)
    bg2v = din('bg2v', [2, 1], F32)
    vones = nc.dram_tensor('vones', [128, 8], FP8, kind='ExternalInput').ap()
    gidx = nc.dram_tensor('gidx', [8, 128], mybir.dt.int32, kind='ExternalInput').ap()
    didx = nc.dram_tensor('didx', [4, 128], mybir.dt.int32, kind='ExternalInput').ap()
    sel4 = din('sel4', [128, 256], BF16)

    out_t = nc.dram_tensor('out_t', [DM, SSL], F32, kind='ExternalOutput').ap()

    with tile.TileContext(nc) as tc:
        with tc.tile_pool(name='consts', bufs=1) as consts, \
             tc.tile_pool(name='pre', bufs=1) as pre, \
             tc.tile_pool(name='dram', bufs=1, space='DRAM') as dram:

            # ---------------- phase A setup: wq + q stream FIRST ----------
            ctx_proj = tc.tile_pool(name='proj', bufs=1)
            proj = ctx_proj.__enter__()
            qt_pair = [proj.tile([128, S], BF16, tag=f'qt_pair{p}',
                                 name=f'qt_pair{p}') for p in range(2)]
            kt_pair = [proj.tile([128, M], BF16, tag=f'kt_pair{p}',
                                 name=f'kt_pair{p}') for p in range(2)]
            # V in fp8, mt-pair subtile layout for DoubleRow:
            # v2_sb[t][ki, 320*j + 80*(2p+h) + c] = V_{mt=2t+j}
            v2_sb = [proj.tile([128, 640], FP8, tag=f'v2_sb{t}',
                               name=f'v2_sb{t}') for t in range(N_MT // 2)]

            ctx_kvw = tc.tile_pool(name='kvw', bufs=1)
            kvw = ctx_kvw.__enter__()
            ctx_qw = tc.tile_pool(name='qw', bufs=1)
            qw = ctx_qw.__enter__()
            ctx_qin = tc.tile_pool(name='qin', bufs=3)
            qin = ctx_qin.__enter__()
            wq_sb = qw.tile([128, 2048], F32R, tag='wq_sb')
            # wq chunk 0 + first q chunk lead the queue so the PE can start
            # as early as possible; the rest of wq follows
            nc.sync.dma_start(out=wq_sb[:, 0:256], in_=wqT[0:128, :])
            qt_chs = []
            for k in range(8):
                qt_ch = qin.tile([128, S], F32R, tag='qt_ch',
                                 name=f'qt_ch{k}')
                nc.sync.dma_start(out=qt_ch[:],
                                  in_=qT[128 * k:128 * (k + 1), :])
                qt_chs.append(qt_ch)
                if k == 0:
                    nc.sync.dma_start(
                        out=wq_sb[:, 256:2048].rearrange('p (k c) -> p k c', k=7),
                        in_=wqT[128:1024, :].rearrange('(k p) c -> p k c', p=128))

            # K/V weights next on the sync queue
            wk_sb = kvw.tile([128, 1536], F32R, tag='wk_sb')
            wv_sb = kvw.tile([128, 1536], F32R, tag='wv_sb')
            nc.sync.dma_start(
                out=wk_sb[:].rearrange('p (k c) -> p k c', k=6),
                in_=wkT[:].rearrange('(k p) c -> p k c', p=128))
            nc.sync.dma_start(
                out=wv_sb[:].rearrange('p (k c) -> p k c', k=6),
                in_=wvT[:].rearrange('(k p) c -> p k c', p=128))

            # ---------------- small constants (gpsimd queue) --------------
            bq_sb = consts.tile([128, 2], F32, tag='bq_sb')
            bk_sb = consts.tile([128, 2], F32, tag='bk_sb')
            for p in range(2):
                nc.gpsimd.dma_start(out=bq_sb[:, p:p + 1], in_=bqv[p:p + 1, :])
                nc.gpsimd.dma_start(out=bk_sb[:, p:p + 1], in_=bkv[p:p + 1, :])
            gidx_sb = []
            for kc in range(8):
                gt = consts.tile([128, 1], mybir.dt.int32, tag=f'gidx{kc}',
                                 name=f'gidx{kc}')
                nc.gpsimd.dma_start(out=gt[:], in_=gidx[kc:kc + 1, :])
                gidx_sb.append(gt)
            didx_sb = []
            for ci in range(4):
                dt_ = consts.tile([128, 1], mybir.dt.int32, tag=f'didx{ci}',
                                  name=f'didx{ci}')
                nc.gpsimd.dma_start(out=dt_[:], in_=didx[ci:ci + 1, :])
                didx_sb.append(dt_)
            sel4_sb = consts.tile([128, 256], BF16, tag='sel4_sb')
            nc.gpsimd.dma_start(out=sel4_sb[:], in_=sel4[:])
            bc0_sb = consts.tile([2, 128], F32R, tag='bc0_sb')
            nc.gpsimd.dma_start(out=bc0_sb[:], in_=bc0[:])
            bo2_sb = consts.tile([128, 8], F32, tag='bo2_sb')
            bg1_sb = consts.tile([128, 8], F32, tag='bg1_sb')
            for k in range(8):
                nc.gpsimd.dma_start(out=bo2_sb[:, k:k + 1], in_=bo2v[k:k + 1, :])
                nc.gpsimd.dma_start(out=bg1_sb[:, k:k + 1], in_=bg1v[k:k + 1, :])
            bg2_sb = consts.tile([2, 1], F32, tag='bg2_sb')
            nc.gpsimd.dma_start(out=bg2_sb[:], in_=bg2v[:])
            wg2_sb = consts.tile([128, 16], F32R, tag='wg2_sb')
            for k in range(8):
                nc.gpsimd.dma_start(out=wg2_sb[:, 2 * k:2 * (k + 1)],
                                    in_=wg2T[128 * k:128 * (k + 1), :])

            # epilogue tiles, filled by big DMAs issued from the scalar
            # queue between chunk pairs (transfers overlap attention)
            wo_bf = pre.tile([128, 8 * DM], FP8, tag='wo_bf')
            wg1_bf = pre.tile([128, 16 * DM], FP8, tag='wg1_bf')
            qs_sb = pre.tile([128, 8 * SSL], F32, tag='qs_sb')
            qs_bf = pre.tile([128, 8 * SSL], FP8, tag='qs_bf')
            # gathered raw retrieved chunks + denominator reciprocals
            rawk = pre.tile([128, 8 * 512], BF16, tag='rawk')
            dgt_all = pre.tile([128, 4 * 512], BF16, tag='dgt_all')
            rdr_all = pre.tile([128, 4 * 512], BF16, tag='rdr_all')

            def preload(step):
                if step >= 3:
                    return
                if step == 0:
                    nc.sync.dma_start(
                        out=qs_sb[:].rearrange('p (k s) -> p k s', k=8),
                        in_=qsT[:].rearrange('(k p) s -> p k s', p=128))
                    nc.sync.dma_start(
                        out=qs_bf[:].rearrange('p (k s) -> p k s', k=8),
                        in_=qsbT[:].rearrange('(k p) s -> p k s', p=128))
                elif step == 1:
                    nc.sync.dma_start(
                        out=wo_bf[:].rearrange('p (k c) -> p k c', k=8),
                        in_=woT[:].rearrange('(k p) c -> p k c', p=128))
                else:
                    nc.sync.dma_start(
                        out=wg1_bf[:].rearrange('p (k c) -> p k c', k=16),
                        in_=wg1T[:].rearrange('(k p) c -> p k c', p=128))

            rt_in = [dram.tile([264, 512], BF16, tag=f'rt_in{i}',
                               name=f'rt_in{i}') for i in range(4)]
            rt_og = [dram.tile([2112, 512], BF16, tag=f'rt_og{i}',
                               name=f'rt_og{i}', addr_space='Shared')
                     for i in range(3)]

            def gather_ci(ci):
                # gather arrival chunk ci (both pairs) + its denominator rows.
                src = rt_og[ci] if ci < 3 else rt_in[3]
                for p in range(2):
                    kc = 2 * ci + p
                    nc.gpsimd.indirect_dma_start(
                        out=rawk[:, 512 * kc:512 * (kc + 1)], out_offset=None,
                        in_=src[:],
                        in_offset=bass.IndirectOffsetOnAxis(ap=gidx_sb[kc][:], axis=0))
                nc.gpsimd.indirect_dma_start(
                    out=dgt_all[:, 512 * ci:512 * (ci + 1)], out_offset=None,
                    in_=src[:],
                    in_offset=bass.IndirectOffsetOnAxis(ap=didx_sb[ci][:], axis=0))

            def recip_ci(ci):
                with nc.allow_low_precision(reason='denominators renormalize'):
                    nc.vector.reciprocal(rdr_all[:, 512 * ci:512 * (ci + 1)],
                                         dgt_all[:, 512 * ci:512 * (ci + 1)])

            # ---------------- phase A: Q projection -----------------------
            ctx_psA = tc.tile_pool(name='psA', bufs=1, space='PSUM')
            psA = ctx_psA.__enter__()
            pqs = [psA.tile([128, 512], F32, tag=f'pq{i}', name=f'pq{i}')
                   for i in range(8)]
            for k in range(8):
                qt_ch = qt_chs[k]
                for p in range(2):
                    for sc in range(N_SC):
                        nc.tensor.matmul(
                            pqs[4 * p + sc][:],
                            wq_sb[:, 256 * k + 128 * p:256 * k + 128 * (p + 1)],
                            qt_ch[:, SC_W * sc:SC_W * (sc + 1)],
                            start=(k == 0), stop=(k == 7))
            for p in range(2):
                for sc in range(N_SC):
                    nc.vector.tensor_scalar_add(
                        qt_pair[p][:, SC_W * sc:SC_W * (sc + 1)],
                        pqs[4 * p + sc][:], bq_sb[:, p:p + 1])
            ctx_psA.__exit__(None, None, None)
            ctx_qin.__exit__(None, None, None)
            ctx_qw.__exit__(None, None, None)

            # ---------------- K/V proj + attention ----------------
            ctx_psAV = tc.tile_pool(name='psAV', bufs=1, space='PSUM')
            psAV = ctx_psAV.__enter__()

            def chunk_pair(sc, a2pool, rtst, psQK):
                for p in range(2):
                    accA = psAV.tile([66, 512], F32, tag='accA')
                    accB = psAV.tile([66, 512], F32, tag='accB')
                    acc = [accA, accB]
                    for t in range(16):  # mt pairs
                        tq = [psQK.tile([128, 1024], F32, tag='tQ',
                                        name=f'tq{sc}{p}{t}{h}')
                              for h in range(2)]
                        # interleave h so QK pairs run concurrently in the
                        # PE's two 64-row groups
                        for k in range(2):
                            mt = 2 * t + k
                            for h in range(2):
                                nc.tensor.matmul(
                                    tq[h][:, 512 * k:512 * (k + 1)],
                                    kt_pair[p][64 * h:64 * (h + 1),
                                               128 * mt:128 * (mt + 1)],
                                    qt_pair[p][64 * h:64 * (h + 1),
                                               SC_W * sc:SC_W * (sc + 1)],
                                    start=True, stop=True)
                        at2s = []
                        for h in range(2):
                            a2 = a2pool.tile([128, 1024], FP8, tag='a2',
                                             name=f'a2{sc}{p}{t}{h}')
                            # ~40% of strips on VectorE (it also carries the
                            # K bias-adds, V copies and epilogue ops)
                            if (2 * t + h) % 5 in (1, 3):
                                nc.vector.tensor_scalar(
                                    a2[:].bitcast(I8), tq[h][:],
                                    SCH8_SCALE, SCH8_MAGIC, ALU.mult, ALU.add)
                            else:
                                nc.scalar.activation(a2[:], tq[h][:], AF.Exp)
                            at2s.append(a2)
                        for h in range(2):
                            nc.tensor.matmul(
                                acc[h][:],
                                v2_sb[t][:].rearrange('p (j c) -> p j c', j=2)
                                    [:, :, 80 * (2 * p + h):80 * (2 * p + h) + 66],
                                at2s[h][:].rearrange('p (j s) -> p j s', j=2),
                                start=(t == 0), stop=(t == 15),
                                perf_mode=PM.DoubleRow)
                    for h in range(2):
                        rt_t = rtst.tile([66, 512], BF16, tag='rt_t',
                                         name=f'rt_t{sc}{p}{h}')
                        nc.scalar.activation(rt_t[:], acc[h][0:66, :], AF.Copy)
                        nc.sync.dma_start(
                            out=rt_in[sc][132 * p + 66 * h:
                                          132 * p + 66 * (h + 1), :],
                            in_=rt_t[:])
                if sc < 3:
                    nc.gpsimd.collective_compute(
                        'AllGather', ALU.bypass,
                        replica_groups=[list(range(NC))],
                        ins=[rt_in[sc][:].opt()],
                        outs=[rt_og[sc][:].opt()])

            ctx_mkin = tc.tile_pool(name='mkin', bufs=1)
            mkin = ctx_mkin.__enter__()
            ctx_psK = tc.tile_pool(name='psK', bufs=1, space='PSUM')
            psK = ctx_psK.__enter__()
            ctx_psV = tc.tile_pool(name='psV', bufs=1, space='PSUM')
            psV = ctx_psV.__enter__()
            for mc in range(8):  # m blocks of 512, k-chunks in half-tiles
                mkb, mvb = [], []
                for hf in range(2):
                    kb = mkin.tile([128, 1536], F32R, tag=f'mkb{hf}',
                                   name=f'mkb{mc}_{hf}')
                    vb = mkin.tile([128, 1536], F32R, tag=f'mvb{hf}',
                                   name=f'mvb{mc}_{hf}')
                    nc.sync.dma_start(
                        out=kb[:].rearrange('p (k s) -> p k s', k=3),
                        in_=mkT[384 * hf:384 * (hf + 1),
                                512 * mc:512 * (mc + 1)].rearrange(
                                    '(k p) s -> p k s', p=128))
                    nc.sync.dma_start(
                        out=vb[:].rearrange('p (k s) -> p k s', k=3),
                        in_=mvT[384 * hf:384 * (hf + 1),
                                512 * mc:512 * (mc + 1)].rearrange(
                                    '(k p) s -> p k s', p=128))
                    mkb.append(kb)
                    mvb.append(vb)
                for p in range(2):
                    pk = psK.tile([128, 512], F32, tag='pk')
                    for k in range(6):
                        nc.tensor.matmul(
                            pk[:],
                            wk_sb[:, 256 * k + 128 * p:256 * k + 128 * (p + 1)],
                            mkb[k // 3][:, 512 * (k % 3):512 * (k % 3 + 1)],
                            start=(k == 0), stop=(k == 5))
                    nc.vector.tensor_scalar_add(
                        kt_pair[p][:, 512 * mc:512 * (mc + 1)], pk[:],
                        bk_sb[:, p:p + 1])
                for ml in range(4):
                    mt = 4 * mc + ml
                    pv = psV.tile([128, 256], F32, tag='pv')
                    for k in range(6):
                        nc.tensor.matmul(
                            pv[:],
                            mvb[k // 3][:, 512 * (k % 3) + 128 * ml:
                                        512 * (k % 3) + 128 * (ml + 1)],
                            wv_sb[:, 256 * k:256 * (k + 1)],
                            start=(k == 0), stop=(k == 5))
                    t2, j2 = mt // 2, mt % 2
                    vh = v2_sb[t2][:, 320 * j2:320 * (j2 + 1)].rearrange(
                        'p (h c) -> p h c', h=4)
                    nc.sync.dma_start(
                        out=vh[:, :, 64:66],
                        in_=vones[:].rearrange('p (h c) -> p h c', h=4))
                    nc.vector.tensor_copy(
                        vh[:, :, 0:64],
                        pv[:].rearrange('p (h d) -> p h d', h=4))

            # chunk pair 0: overlapped with K/V projection (2 psQK bufs)
            ctx_attnA = tc.tile_pool(name='attnA', bufs=4)
            apoolA = ctx_attnA.__enter__()
            ctx_rtstA = tc.tile_pool(name='rtstA', bufs=2)
            rtstA = ctx_rtstA.__enter__()
            ctx_psQKA = tc.tile_pool(name='psQKA', bufs=2, space='PSUM')
            psQKA = ctx_psQKA.__enter__()
            chunk_pair(0, apoolA, rtstA, psQKA)
            preload(0)
            ctx_psQKA.__exit__(None, None, None)
            ctx_rtstA.__exit__(None, None, None)
            ctx_attnA.__exit__(None, None, None)
            ctx_psV.__exit__(None, None, None)
            ctx_psK.__exit__(None, None, None)
            ctx_mkin.__exit__(None, None, None)
            ctx_kvw.__exit__(None, None, None)

            # pairs 1-3: 3 psQK bufs
            with tc.tile_pool(name='attnB', bufs=6) as apoolB, \
                 tc.tile_pool(name='rtstB', bufs=6) as rtstB, \
                 tc.tile_pool(name='psQKB', bufs=3, space='PSUM') as psQKB:
                for sc in range(1, N_SC):
                    chunk_pair(sc, apoolB, rtstB, psQKB)
                    preload(sc)
                    # AG(sc-2) completed during chunk sc-1, so this gather
                    # and reciprocal never block the queues mid-attention
                    if sc >= 2:
                        gather_ci(sc - 2)
                        recip_ci(sc - 2)
            ctx_psAV.__exit__(None, None, None)
            ctx_proj.__exit__(None, None, None)

            # ---------------- epilogue (own s-slice) ----------------
            with tc.tile_pool(name='ep', bufs=1) as ep, \
                 tc.tile_pool(name='ept', bufs=3) as ept:
                rtn = ep.tile([128, 8 * 512], FP8, tag='rtn')
                rtn2 = rtn[:].rearrange('p (kc s) -> p kc s', kc=8)
                wo8v = wo_bf[:].rearrange('p (kc c) -> p kc c', kc=8)
                wg18v = wg1_bf[:].rearrange('p (kc c) -> p kc c', kc=16)
                qs8v = qs_bf[:].rearrange('p (kc s) -> p kc s', kc=8)

                def normalize_ci(ci, psN):
                    for p in range(2):
                        kc = 2 * ci + p
                        bcp = psN.tile([128, 512], F32, tag='bcp',
                                       name=f'bcp{ci}{p}')
                        nc.tensor.matmul(bcp[:],
                                         sel4_sb[:, 128 * p:128 * (p + 1)],
                                         rdr_all[:, 512 * ci:512 * (ci + 1)],
                                         start=True, stop=True)
                        nc.vector.tensor_tensor(
                            rtn[:, 512 * kc:512 * (kc + 1)],
                            rawk[:, 512 * kc:512 * (kc + 1)], bcp[:], ALU.mult)

                # chunks 2 (AG done during chunk 3) and 3 (own slice, no AG)
                gather_ci(2)
                recip_ci(2)
                gather_ci(3)
                recip_ci(3)
                ctx_psN = tc.tile_pool(name='psN', bufs=2, space='PSUM')
                psN = ctx_psN.__enter__()
                for ci in range(3):
                    normalize_ci(ci, psN)

                gq_sb = ep.tile([128, 8 * 512], F32, tag='gq_sb')
                # Wg1 @ q for both hidden halves — no AG dependency
                for half in range(2):
                    with tc.tile_pool(name=f'psGQ{half}', bufs=1,
                                      space='PSUM') as psGQ:
                        pgq = [psGQ.tile([128, 512], F32, tag=f'pgq{i}',
                                         name=f'pgq{half}_{i}') for i in range(4)]
                        for cj in range(4):
                            for i in range(4):
                                dt = 4 * half + i
                                nc.tensor.matmul(
                                    pgq[i][:],
                                    wg18v[:, 2 * cj:2 * cj + 2,
                                          128 * dt:128 * (dt + 1)],
                                    qs8v[:, 2 * cj:2 * cj + 2, :],
                                    start=(cj == 0), stop=(cj == 3),
                                    perf_mode=PM.DoubleRow)
                        for i in range(4):
                            dt = 4 * half + i
                            nc.vector.tensor_copy(
                                gq_sb[:, 512 * dt:512 * (dt + 1)], pgq[i][:])

                # normalize the own-slice chunk (recip ran during Wg1@q)
                normalize_ci(3, psN)
                ctx_psN.__exit__(None, None, None)

                # Wo projection interleaved with Wg1@o half 0
                oT = ep.tile([128, 8 * SSL], BF16, tag='oT')
                oT8 = ep.tile([128, 8 * SSL], FP8, tag='oT8')
                oT8v = oT8[:].rearrange('p (dt s) -> p dt s', dt=8)
                sl = ep.tile([128, 8 * 512], F32R, tag='sl')

                def silu_block(pg, dt):
                    hg = ept.tile([128, 512], F32, tag='hg', name=f'hg{dt}')
                    nc.vector.tensor_tensor(
                        hg[:], pg[:], gq_sb[:, 512 * dt:512 * (dt + 1)], ALU.add)
                    sg = ept.tile([128, 512], F32, tag='sg', name=f'sg{dt}')
                    nc.scalar.activation(sg[:], hg[:], AF.Sigmoid,
                                         bias=bg1_sb[:, dt:dt + 1])
                    gg = ept.tile([128, 512], F32, tag='gg', name=f'gg{dt}')
                    nc.vector.tensor_scalar_add(gg[:], hg[:], bg1_sb[:, dt:dt + 1])
                    nc.vector.tensor_tensor(
                        sl[:, 512 * dt:512 * (dt + 1)], gg[:], sg[:], ALU.mult)

                def wg1o_step(pgo, half, cpair, start, stop):
                    for i in range(4):
                        nc.tensor.matmul(
                            pgo[i][:],
                            wg18v[:, 8 + 2 * cpair:8 + 2 * cpair + 2,
                                  512 * half + 128 * i:512 * half + 128 * (i + 1)],
                            oT8v[:, 2 * cpair:2 * cpair + 2, :],
                            start=start, stop=stop,
                            perf_mode=PM.DoubleRow)

                ctx_psG0 = tc.tile_pool(name='psG0', bufs=1, space='PSUM')
                psG0 = ctx_psG0.__enter__()
                pgo0 = [psG0.tile([128, 512], F32, tag=f'pgo0_{i}',
                                  name=f'pgo0_{i}') for i in range(4)]
                ctx_psWo = tc.tile_pool(name='psWo', bufs=2, space='PSUM')
                psWo = ctx_psWo.__enter__()
                for dt in range(8):
                    po = psWo.tile([128, 512], F32, tag='po')
                    for cj in range(4):
                        nc.tensor.matmul(
                            po[:], wo8v[:, 2 * cj:2 * cj + 2,
                                        128 * dt:128 * (dt + 1)],
                            rtn2[:, 2 * cj:2 * cj + 2, :],
                            start=(cj == 0), stop=(cj == 3),
                            perf_mode=PM.DoubleRow)
                    nc.vector.tensor_scalar_add(
                        oT[:, SSL * dt:SSL * (dt + 1)], po[:], bo2_sb[:, dt:dt + 1])
                    nc.scalar.activation(oT8[:, SSL * dt:SSL * (dt + 1)],
                                         po[:], AF.Copy, scale=1.0 / 16.0)
                    # Wg1 @ o (half 0) lags so PE never waits on the oT8
                    # ACT copies; pair c ready after dt=2c+1
                    if dt >= 3 and dt % 2 == 1:
                        wg1o_step(pgo0, 0, (dt - 3) // 2,
                                  start=(dt == 3), stop=False)
                ctx_psWo.__exit__(None, None, None)
                wg1o_step(pgo0, 0, 2, start=False, stop=False)
                wg1o_step(pgo0, 0, 3, start=False, stop=True)
                for i in range(4):
                    silu_block(pgo0[i], i)
                ctx_psG0.__exit__(None, None, None)

                with tc.tile_pool(name='psG1', bufs=1, space='PSUM') as psG1:
                    pgo1 = [psG1.tile([128, 512], F32, tag=f'pgo1_{i}',
                                      name=f'pgo1_{i}') for i in range(4)]
                    for cpair in range(4):
                        wg1o_step(pgo1, 1, cpair,
                                  start=(cpair == 0), stop=(cpair == 3))
                    for i in range(4):
                        silu_block(pgo1[i], 4 + i)

                # gate scalar: sigmoid(Wg2 @ sl + bg2), broadcast to 128 rows
                with tc.tile_pool(name='psT', bufs=1, space='PSUM') as psT:
                    pgt = psT.tile([2, 512], F32, tag='pgt')
                    for kc in range(8):
                        nc.tensor.matmul(pgt[:], wg2_sb[:, 2 * kc:2 * (kc + 1)],
                                         sl[:, 512 * kc:512 * (kc + 1)],
                                         start=(kc == 0), stop=(kc == 7))
                    gate = ep.tile([2, 512], F32R, tag='gate')
                    nc.scalar.activation(gate[:], pgt[:], AF.Sigmoid, bias=bg2_sb[:])
                    gb = psT.tile([128, 512], F32, tag='gb')
                    nc.tensor.matmul(gb[:], bc0_sb[:], gate[:], start=True, stop=True)
                    gbs = ep.tile([128, 512], F32, tag='gbs')
                    nc.vector.tensor_copy(gbs[:], gb[:])

                    # out = q + gate * o
                    for dt in range(8):
                        go = ept.tile([128, 512], F32, tag='go')
                        nc.vector.tensor_tensor(
                            go[:], gbs[:], oT[:, SSL * dt:SSL * (dt + 1)], ALU.mult)
                        fo = ept.tile([128, 512], F32, tag='fo')
                        nc.vector.tensor_tensor(
                            fo[:], go[:], qs_sb[:, SSL * dt:SSL * (dt + 1)], ALU.add)
                        nc.sync.dma_start(out=out_t[128 * dt:128 * (dt + 1), :],
                                          in_=fo[:])

    nc.compile()
    return nc


def _shard(inputs):
    import ml_dtypes
    _bf16 = ml_dtypes.bfloat16
    _fp8 = ml_dtypes.float8_e4m3fn
    q = np.asarray(inputs['query'], np.float32)
    mk = np.asarray(inputs['memory_keys'], np.float32)
    mv = np.asarray(inputs['memory_values'], np.float32)
    Wq = np.asarray(inputs['Wq'], np.float32); bq = np.asarray(inputs['bq'], np.float32)
    Wk = np.asarray(inputs['Wk'], np.float32); bk = np.asarray(inputs['bk'], np.float32)
    Wv = np.asarray(inputs['Wv'], np.float32); bv = np.asarray(inputs['bv'], np.float32)
    Wo = np.asarray(inputs['Wo'], np.float32); bo = np.asarray(inputs['bo'], np.float32)
    Wg1 = np.asarray(inputs['Wg1'], np.float32); bg1 = np.asarray(inputs['bg1'], np.float32)
    Wg2 = np.asarray(inputs['Wg2'], np.float32); bg2 = np.asarray(inputs['bg2'], np.float32)

    scale = Dh ** -0.5
    bo2 = bo + Wo @ bv
    # the o-half of Wg1 sees oT8 = o - bo2, so fold Wg1o @ bo2 into bg1
    bg1_eff = bg1 + Wg1[:, DM:] @ bo2
    bc0 = np.zeros((2, 128), np.float32)
    bc0[0, :] = 1.0
    wg2T = np.zeros((DM, 2), np.float32)
    wg2T[:, 0] = Wg2[0]
    bg2v = np.zeros((2, 1), np.float32)
    bg2v[:, 0] = bg2[0]
    # sel4[2p + j//64, 128p + j] = 1/16 — picks denominator-recip row
    # 2p+h and pre-scales rtn into fp8 range (undone by the 16x in Wo)
    _sel4 = np.zeros((128, 256), np.float32)
    for _p in range(2):
        for _j in range(128):
            _sel4[2 * _p + _j // 64, 128 * _p + _j] = 1.0 / 16.0

    qT_b = [np.ascontiguousarray(q[b].T) for b in range(B)]
    mkT_b = [np.ascontiguousarray(mk[b].T) for b in range(B)]
    mvT_b = [np.ascontiguousarray(mv[b].T) for b in range(B)]
    WoT = np.ascontiguousarray(Wo.T)     # [1024 in, 1024 out]
    Wg1T = np.ascontiguousarray(Wg1.T)   # [2048 in, 1024 out]

    in_maps = []
    for c in range(NC):
        b, g = c // GS, c % GS
        hs = slice(64 * 4 * g, 64 * (4 * g + 4))  # rows of W for this core's 4 heads
        # s-rotation: compile chunk i processes logical slice (g+1+i)%4
        lsl = [(g + 1 + i) % 4 for i in range(4)]
        qT_c = np.concatenate([qT_b[b][:, 512 * l:512 * (l + 1)] for l in lsl],
                              axis=1)
        # arrival chunk ci comes from group-rank r_i = (g-1-ci)%4; its pair-p
        # block maps to Wo/Wg1 input-channel block 2*r_i + p
        ch = [2 * ((g - 1 - ci) % 4) + p for ci in range(4) for p in range(2)]
        woT_c = np.concatenate([WoT[128 * cb:128 * (cb + 1), :] for cb in ch])
        wg1T_c = np.concatenate(
            [Wg1T[0:1024, :]]
            + [Wg1T[1024 + 128 * cb:1024 + 128 * (cb + 1), :] for cb in ch])
        # gather row of (ci, p, head h, dim d):
        #   ci<3: 264*(4b + r_i) + 132p + 66h + d   in rt_og[ci]
        #   ci=3: 264*3        + 132p + 66h + d     in rt_in (own slice)
        def _base(ci, p):
            if ci < 3:
                return 264 * (4 * b + (g - 1 - ci) % 4) + 132 * p
            return 132 * p
        _gidx = np.asarray(
            [[_base(kc // 2, kc % 2) + 66 * (j // 64) + (j % 64)
              for j in range(128)] for kc in range(8)], np.int32)
        # denominator rows: j = 2p + h (4 valid); junk rows point at a
        # denominator row too (never zero, keeps 1/x finite)
        _didx = np.asarray(
            [[_base(ci, (j // 2) % 2) + 66 * (j % 2) + 64 if j < 4
              else _base(ci, 0) + 64 for j in range(128)]
             for ci in range(4)], np.int32)
        qs_c = np.ascontiguousarray(q[b].T[:, SSL * g:SSL * (g + 1)])
        in_maps.append({
            'qT': np.ascontiguousarray(qT_c),
            'mkT': mkT_b[b],
            'mvT': mvT_b[b],
            'wqT': np.ascontiguousarray((Wq[hs] * scale).T),
            'wkT': np.ascontiguousarray(Wk[hs].T),
            'wvT': np.ascontiguousarray(Wv[hs].T),
            'woT': np.ascontiguousarray((woT_c * 16.0).astype(_fp8)),
            'wg1T': np.ascontiguousarray((wg1T_c * 16.0).astype(_fp8)),
            'wg2T': wg2T,
            'qsT': qs_c,
            'qsbT': np.ascontiguousarray((qs_c / 16.0).astype(_fp8)),
            'vones': np.ascontiguousarray(np.tile([1.0, 0.0], 4)[None, :].repeat(128, 0).astype(_fp8)),
            'bc0': bc0,
            'bqv': np.ascontiguousarray((bq[hs] * scale).reshape(2, 128)),
            'bkv': np.ascontiguousarray(bk[hs].reshape(2, 128)),
            'bo2v': np.ascontiguousarray(bo2.reshape(8, 128)),
            'bg1v': np.ascontiguousarray(bg1_eff.reshape(8, 128)),
            'bg2v': bg2v,
            'gidx': _gidx,
            'didx': _didx,
            'sel4': np.ascontiguousarray(_sel4.astype(_bf16)),
        })
    return in_maps


def _run(inputs, trace=False):
    global _PROG
    from concourse.bass_utils import run_bass_kernel_spmd
    if _PROG is None:
        _PROG = _build_program()
    in_maps = _shard(inputs)
    res = run_bass_kernel_spmd(_PROG, in_maps, list(range(NC)), trace=trace)
    out = np.empty((B, S, DM), np.float32)
    for c in range(NC):
        b, g = c // GS, c % GS
        out[b, SSL * g:SSL * (g + 1), :] = res.results[c]['out_t'].T
    return out, res


def kernel(**inputs) -> np.ndarray:
    out, _ = _run(inputs, trace=False)
    return out


# revision 33
# speedup vs baseline: 1.0460x; 1.0460x over previous
"""AdvancedVectorMemory fused kernel for 8 Trainium2 NeuronCores.

Sharding: core c handles batch b = c//4 and heads 4*(c%4) .. 4*(c%4)+3
(data parallel over batch, tensor parallel over heads). Attention runs
flash-style per head pair with fused denominators (ones column in V).

Perf structure:
 - s-rotation: core (b, g) processes logical s-slices in the order
   g+1, g+2, g+3, g (mod 4), host-side permutation of q columns. Its
   own slice is computed LAST, so only 3 AllGathers are needed; each AG
   overlaps the next chunk pair's compute.
 - exp is split across TWO engines: even strips use ScalarE's real Exp;
   odd strips use a Schraudolph bit-trick on VectorE (logit*184.665 +
   magic constant in f32; the low 16 bits of the f32 sum ARE the bf16
   bit pattern of ~exp(logit), read back via bitcast + stride-2 AP).
   Softmax renormalization absorbs the +-3% systematic error.
 - rt (attention accumulator) copies run on ScalarE, freeing VectorE
   for the magic-exp strips.
 - Wo / Wg1 ship from host as bf16 (half the DMA, no on-chip casts);
   qs ships both f32 (residual) and bf16 (matmul).
 - Startup: wq + first q chunks DMA first; small consts go to the
   gpsimd queue; big preloads issue from the scalar queue mid-attention.
 - AllGather outputs are Shared-scratchpad DRAM (fast HBM-HBM path).
 - Gathers + denominator reciprocals run during attention; the epilogue
   normalizes early chunks first so Wo can start immediately.
"""
import sys
import numpy as np

for _p in ('/opt/trn_rl_repo', '/root/.axon_site/_ro/trn_rl_repo'):
    if _p not in sys.path:
        sys.path.insert(0, _p)

B, S, M = 2, 2048, 4096
DM, DK = 1024, 768
H, Dh = 16, 64
NC = 8
GS = 4           # group size (cores per batch)
SC_W = 512       # s-chunk width
N_SC = S // SC_W
N_MT = M // 128  # 32 m-tiles
SSL = S // GS    # per-core s-slice for the epilogue (512)

# Schraudolph fp8e4m3 exp-by-bits on VectorE: int8 convert of
# x*(8/ln2) + (56 - c); the int8 bits ARE the fp8 pattern of ~exp(x).
# The +-3% systematic error renormalizes out in the softmax.
SCH8_SCALE = 8.0 / float(np.log(2.0))
SCH8_MAGIC = 56.0 - 0.344


_PROG = None


def _build_program():
    from concourse import bacc, mybir, tile
    import concourse.bass as bass

    F32 = mybir.dt.float32
    F32R = mybir.dt.float32r
    BF16 = mybir.dt.bfloat16
    FP8 = mybir.dt.float8e4
    I8 = mybir.dt.int8
    AF = mybir.ActivationFunctionType
    ALU = mybir.AluOpType
    PM = mybir.MatmulPerfMode

    nc = bacc.Bacc('TRN2', target_bir_lowering=False, debug=False, num_devices=NC)

    def din(name, shape, dt=F32R):
        return nc.dram_tensor(name, shape, dt, kind='ExternalInput').ap()

    qT = din('qT', [DM, S])
    mkT = din('mkT', [DK, M])
    mvT = din('mvT', [DK, M])
    wqT = din('wqT', [DM, 256])
    wkT = din('wkT', [DK, 256])
    wvT = din('wvT', [DK, 256])
    woT = din('woT', [DM, DM], FP8)        # 16*Wo
    wg1T = din('wg1T', [2 * DM, DM], FP8)  # 16*Wg1
    wg2T = din('wg2T', [DM, 2])
    qsT = din('qsT', [DM, SSL], F32)
    qsbT = din('qsbT', [DM, SSL], FP8)     # qs/16
    bc0 = din('bc0', [2, 128])        # row0 = ones (gate broadcast)
    bqv = din('bqv', [2, 128], F32)
    bkv = din('bkv', [2, 128], F32)
    bo2v = din('bo2v', [8, 128], F32)
    bg1v = din('bg1v', [8, 128], F32)
    bg2v = din('bg2v', [2, 1], F32)
    vones = nc.dram_tensor('vones', [128, 8], FP8, kind='ExternalInput').ap()
    gidx = nc.dram_tensor('gidx', [8, 128], mybir.dt.int32, kind='ExternalInput').ap()
    didx = nc.dram_tensor('didx', [4, 128], mybir.dt.int32, kind='ExternalInput').ap()
    sel4 = din('sel4', [128, 256], BF16)

    out_t = nc.dram_tensor('out_t', [DM, SSL], F32, kind='ExternalOutput').ap()

    with tile.TileContext(nc) as tc:
        with tc.tile_pool(name='consts', bufs=1) as consts, \
             tc.tile_pool(name='pre', bufs=1) as pre, \
             tc.tile_pool(name='dram', bufs=1, space='DRAM') as dram:

            # ---------------- phase A setup: wq + q stream FIRST ----------
            ctx_proj = tc.tile_pool(name='proj', bufs=1)
            proj = ctx_proj.__enter__()
            qt_pair = [proj.tile([128, S], BF16, tag=f'qt_pair{p}',
                                 name=f'qt_pair{p}') for p in range(2)]
            kt_pair = [proj.tile([128, M], BF16, tag=f'kt_pair{p}',
                                 name=f'kt_pair{p}') for p in range(2)]
            # V in fp8, mt-pair subtile layout for DoubleRow:
            # v2_sb[t][ki, 320*j + 80*(2p+h) + c] = V_{mt=2t+j}
            v2_sb = [proj.tile([128, 640], FP8, tag=f'v2_sb{t}',
                               name=f'v2_sb{t}') for t in range(N_MT // 2)]

            ctx_kvw = tc.tile_pool(name='kvw', bufs=1)
            kvw = ctx_kvw.__enter__()
            ctx_qw = tc.tile_pool(name='qw', bufs=1)
            qw = ctx_qw.__enter__()
            ctx_qin = tc.tile_pool(name='qin', bufs=3)
            qin = ctx_qin.__enter__()
            wq_sb = qw.tile([128, 2048], F32R, tag='wq_sb')
            # wq chunk 0 + first q chunk lead the queue so the PE can start
            # as early as possible; the rest of wq follows
            nc.sync.dma_start(out=wq_sb[:, 0:256], in_=wqT[0:128, :])
            qt_chs = []
            for k in range(8):
                qt_ch = qin.tile([128, S], F32R, tag='qt_ch',
                                 name=f'qt_ch{k}')
                nc.sync.dma_start(out=qt_ch[:],
                                  in_=qT[128 * k:128 * (k + 1), :])
                qt_chs.append(qt_ch)
                if k == 0:
                    nc.sync.dma_start(
                        out=wq_sb[:, 256:2048].rearrange('p (k c) -> p k c', k=7),
                        in_=wqT[128:1024, :].rearrange('(k p) c -> p k c', p=128))

            # K/V weights next on the sync queue
            wk_sb = kvw.tile([128, 1536], F32R, tag='wk_sb')
            wv_sb = kvw.tile([128, 1536], F32R, tag='wv_sb')
            nc.sync.dma_start(
                out=wk_sb[:].rearrange('p (k c) -> p k c', k=6),
                in_=wkT[:].rearrange('(k p) c -> p k c', p=128))
            nc.sync.dma_start(
                out=wv_sb[:].rearrange('p (k c) -> p k c', k=6),
                in_=wvT[:].rearrange('(k p) c -> p k c', p=128))

            # ---------------- small constants (gpsimd queue) --------------
            bq_sb = consts.tile([128, 2], F32, tag='bq_sb')
            bk_sb = consts.tile([128, 2], F32, tag='bk_sb')
            for p in range(2):
                nc.gpsimd.dma_start(out=bq_sb[:, p:p + 1], in_=bqv[p:p + 1, :])
                nc.gpsimd.dma_start(out=bk_sb[:, p:p + 1], in_=bkv[p:p + 1, :])
            gidx_sb = []
            for kc in range(8):
                gt = consts.tile([128, 1], mybir.dt.int32, tag=f'gidx{kc}',
                                 name=f'gidx{kc}')
                nc.gpsimd.dma_start(out=gt[:], in_=gidx[kc:kc + 1, :])
                gidx_sb.append(gt)
            didx_sb = []
            for ci in range(4):
                dt_ = consts.tile([128, 1], mybir.dt.int32, tag=f'didx{ci}',
                                  name=f'didx{ci}')
                nc.gpsimd.dma_start(out=dt_[:], in_=didx[ci:ci + 1, :])
                didx_sb.append(dt_)
            sel4_sb = consts.tile([128, 256], BF16, tag='sel4_sb')
            nc.gpsimd.dma_start(out=sel4_sb[:], in_=sel4[:])
            bc0_sb = consts.tile([2, 128], F32R, tag='bc0_sb')
            nc.gpsimd.dma_start(out=bc0_sb[:], in_=bc0[:])
            bo2_sb = consts.tile([128, 8], F32, tag='bo2_sb')
            bg1_sb = consts.tile([128, 8], F32, tag='bg1_sb')
            for k in range(8):
                nc.gpsimd.dma_start(out=bo2_sb[:, k:k + 1], in_=bo2v[k:k + 1, :])
                nc.gpsimd.dma_start(out=bg1_sb[:, k:k + 1], in_=bg1v[k:k + 1, :])
            bg2_sb = consts.tile([2, 1], F32, tag='bg2_sb')
            nc.gpsimd.dma_start(out=bg2_sb[:], in_=bg2v[:])
            wg2_sb = consts.tile([128, 16], F32R, tag='wg2_sb')
            for k in range(8):
                nc.gpsimd.dma_start(out=wg2_sb[:, 2 * k:2 * (k + 1)],
                                    in_=wg2T[128 * k:128 * (k + 1), :])

            # epilogue tiles, filled by big DMAs issued from the scalar
            # queue between chunk pairs (transfers overlap attention)
            wo_bf = pre.tile([128, 8 * DM], FP8, tag='wo_bf')
            wg1_bf = pre.tile([128, 16 * DM], FP8, tag='wg1_bf')
            qs_sb = pre.tile([128, 8 * SSL], F32, tag='qs_sb')
            qs_bf = pre.tile([128, 8 * SSL], FP8, tag='qs_bf')
            # gathered raw retrieved chunks + denominator reciprocals
            rawk = pre.tile([128, 8 * 512], BF16, tag='rawk')
            dgt_all = pre.tile([128, 4 * 512], BF16, tag='dgt_all')
            rdr_all = pre.tile([128, 4 * 512], BF16, tag='rdr_all')

            def preload(step):
                if step >= 3:
                    return
                if step == 0:
                    nc.sync.dma_start(
                        out=qs_sb[:].rearrange('p (k s) -> p k s', k=8),
                        in_=qsT[:].rearrange('(k p) s -> p k s', p=128))
                    nc.sync.dma_start(
                        out=qs_bf[:].rearrange('p (k s) -> p k s', k=8),
                        in_=qsbT[:].rearrange('(k p) s -> p k s', p=128))
                elif step == 1:
                    nc.sync.dma_start(
                        out=wo_bf[:].rearrange('p (k c) -> p k c', k=8),
                        in_=woT[:].rearrange('(k p) c -> p k c', p=128))
                else:
                    nc.sync.dma_start(
                        out=wg1_bf[:].rearrange('p (k c) -> p k c', k=16),
                        in_=wg1T[:].rearrange('(k p) c -> p k c', p=128))

            rt_in = [dram.tile([264, 512], BF16, tag=f'rt_in{i}',
                               name=f'rt_in{i}') for i in range(4)]
            rt_og = [dram.tile([2112, 512], BF16, tag=f'rt_og{i}',
                               name=f'rt_og{i}', addr_space='Shared')
                     for i in range(3)]

            def gather_ci(ci):
                # gather arrival chunk ci (both pairs) + its denominator rows.
                src = rt_og[ci] if ci < 3 else rt_in[3]
                for p in range(2):
                    kc = 2 * ci + p
                    nc.gpsimd.indirect_dma_start(
                        out=rawk[:, 512 * kc:512 * (kc + 1)], out_offset=None,
                        in_=src[:],
                        in_offset=bass.IndirectOffsetOnAxis(ap=gidx_sb[kc][:], axis=0))
                nc.gpsimd.indirect_dma_start(
                    out=dgt_all[:, 512 * ci:512 * (ci + 1)], out_offset=None,
                    in_=src[:],
                    in_offset=bass.IndirectOffsetOnAxis(ap=didx_sb[ci][:], axis=0))

            def recip_ci(ci):
                with nc.allow_low_precision(reason='denominators renormalize'):
                    nc.vector.reciprocal(rdr_all[:, 512 * ci:512 * (ci + 1)],
                                         dgt_all[:, 512 * ci:512 * (ci + 1)])

            # ---------------- phase A: Q projection -----------------------
            ctx_psA = tc.tile_pool(name='psA', bufs=1, space='PSUM')
            psA = ctx_psA.__enter__()
            pqs = [psA.tile([128, 512], F32, tag=f'pq{i}', name=f'pq{i}')
                   for i in range(8)]
            for k in range(8):
                qt_ch = qt_chs[k]
                for p in range(2):
                    for sc in range(N_SC):
                        nc.tensor.matmul(
                            pqs[4 * p + sc][:],
                            wq_sb[:, 256 * k + 128 * p:256 * k + 128 * (p + 1)],
                            qt_ch[:, SC_W * sc:SC_W * (sc + 1)],
                            start=(k == 0), stop=(k == 7))
            for p in range(2):
                for sc in range(N_SC):
                    nc.vector.tensor_scalar_add(
                        qt_pair[p][:, SC_W * sc:SC_W * (sc + 1)],
                        pqs[4 * p + sc][:], bq_sb[:, p:p + 1])
            ctx_psA.__exit__(None, None, None)
            ctx_qin.__exit__(None, None, None)
            ctx_qw.__exit__(None, None, None)

            # ---------------- K/V proj + attention ----------------
            ctx_psAV = tc.tile_pool(name='psAV', bufs=1, space='PSUM')
            psAV = ctx_psAV.__enter__()

            def chunk_pair(sc, a2pool, rtst, psQK):
                for p in range(2):
                    accA = psAV.tile([66, 512], F32, tag='accA')
                    accB = psAV.tile([66, 512], F32, tag='accB')
                    acc = [accA, accB]
                    for t in range(16):  # mt pairs
                        tq = [psQK.tile([128, 1024], F32, tag='tQ',
                                        name=f'tq{sc}{p}{t}{h}')
                              for h in range(2)]
                        # interleave h so QK pairs run concurrently in the
                        # PE's two 64-row groups
                        for k in range(2):
                            mt = 2 * t + k
                            for h in range(2):
                                nc.tensor.matmul(
                                    tq[h][:, 512 * k:512 * (k + 1)],
                                    kt_pair[p][64 * h:64 * (h + 1),
                                               128 * mt:128 * (mt + 1)],
                                    qt_pair[p][64 * h:64 * (h + 1),
                                               SC_W * sc:SC_W * (sc + 1)],
                                    start=True, stop=True)
                        at2s = []
                        for h in range(2):
                            a2 = a2pool.tile([128, 1024], FP8, tag='a2',
                                             name=f'a2{sc}{p}{t}{h}')
                            # exactly one ACT and one DVE strip per t-group:
                            # an even supply rhythm keeps the PE dense (an
                            # uneven split makes the PE micro-idle and the
                            # HAM clock-gate oscillate to half speed)
                            # exactly one ACT and one DVE strip per
                            # t-group: an even supply rhythm keeps the PE
                            # dense (uneven splits make it micro-idle and
                            # the HAM clock-gate oscillate to half speed)
                            if (t + h) % 2 == 0:
                                nc.scalar.activation(a2[:], tq[h][:], AF.Exp)
                            else:
                                nc.vector.tensor_scalar(
                                    a2[:].bitcast(I8), tq[h][:],
                                    SCH8_SCALE, SCH8_MAGIC, ALU.mult, ALU.add)
                            at2s.append(a2)
                        for h in range(2):
                            nc.tensor.matmul(
                                acc[h][:],
                                v2_sb[t][:].rearrange('p (j c) -> p j c', j=2)
                                    [:, :, 80 * (2 * p + h):80 * (2 * p + h) + 66],
                                at2s[h][:].rearrange('p (j s) -> p j s', j=2),
                                start=(t == 0), stop=(t == 15),
                                perf_mode=PM.DoubleRow)
                    for h in range(2):
                        rt_t = rtst.tile([66, 512], BF16, tag='rt_t',
                                         name=f'rt_t{sc}{p}{h}')
                        nc.scalar.activation(rt_t[:], acc[h][0:66, :], AF.Copy)
                        nc.sync.dma_start(
                            out=rt_in[sc][132 * p + 66 * h:
                                          132 * p + 66 * (h + 1), :],
                            in_=rt_t[:])
                if sc < 3:
                    nc.gpsimd.collective_compute(
                        'AllGather', ALU.bypass,
                        replica_groups=[list(range(NC))],
                        ins=[rt_in[sc][:].opt()],
                        outs=[rt_og[sc][:].opt()])

            ctx_mkin = tc.tile_pool(name='mkin', bufs=1)
            mkin = ctx_mkin.__enter__()
            ctx_psK = tc.tile_pool(name='psK', bufs=1, space='PSUM')
            psK = ctx_psK.__enter__()
            ctx_psV = tc.tile_pool(name='psV', bufs=1, space='PSUM')
            psV = ctx_psV.__enter__()
            for mc in range(8):  # m blocks of 512, k-chunks in half-tiles
                mkb, mvb = [], []
                for hf in range(2):
                    kb = mkin.tile([128, 1536], F32R, tag=f'mkb{hf}',
                                   name=f'mkb{mc}_{hf}')
                    vb = mkin.tile([128, 1536], F32R, tag=f'mvb{hf}',
                                   name=f'mvb{mc}_{hf}')
                    nc.sync.dma_start(
                        out=kb[:].rearrange('p (k s) -> p k s', k=3),
                        in_=mkT[384 * hf:384 * (hf + 1),
                                512 * mc:512 * (mc + 1)].rearrange(
                                    '(k p) s -> p k s', p=128))
                    nc.sync.dma_start(
                        out=vb[:].rearrange('p (k s) -> p k s', k=3),
                        in_=mvT[384 * hf:384 * (hf + 1),
                                512 * mc:512 * (mc + 1)].rearrange(
                                    '(k p) s -> p k s', p=128))
                    mkb.append(kb)
                    mvb.append(vb)
                for p in range(2):
                    pk = psK.tile([128, 512], F32, tag='pk')
                    for k in range(6):
                        nc.tensor.matmul(
                            pk[:],
                            wk_sb[:, 256 * k + 128 * p:256 * k + 128 * (p + 1)],
                            mkb[k // 3][:, 512 * (k % 3):512 * (k % 3 + 1)],
                            start=(k == 0), stop=(k == 5))
                    nc.vector.tensor_scalar_add(
                        kt_pair[p][:, 512 * mc:512 * (mc + 1)], pk[:],
                        bk_sb[:, p:p + 1])
                for ml in range(4):
                    mt = 4 * mc + ml
                    pv = psV.tile([128, 256], F32, tag='pv')
                    for k in range(6):
                        nc.tensor.matmul(
                            pv[:],
                            mvb[k // 3][:, 512 * (k % 3) + 128 * ml:
                                        512 * (k % 3) + 128 * (ml + 1)],
                            wv_sb[:, 256 * k:256 * (k + 1)],
                            start=(k == 0), stop=(k == 5))
                    t2, j2 = mt // 2, mt % 2
                    vh = v2_sb[t2][:, 320 * j2:320 * (j2 + 1)].rearrange(
                        'p (h c) -> p h c', h=4)
                    nc.sync.dma_start(
                        out=vh[:, :, 64:66],
                        in_=vones[:].rearrange('p (h c) -> p h c', h=4))
                    nc.scalar.activation(
                        vh[:, :, 0:64],
                        pv[:].rearrange('p (h d) -> p h d', h=4), AF.Copy)

            # chunk pair 0: overlapped with K/V projection (2 psQK bufs)
            ctx_attnA = tc.tile_pool(name='attnA', bufs=4)
            apoolA = ctx_attnA.__enter__()
            ctx_rtstA = tc.tile_pool(name='rtstA', bufs=2)
            rtstA = ctx_rtstA.__enter__()
            ctx_psQKA = tc.tile_pool(name='psQKA', bufs=2, space='PSUM')
            psQKA = ctx_psQKA.__enter__()
            chunk_pair(0, apoolA, rtstA, psQKA)
            preload(0)
            ctx_psQKA.__exit__(None, None, None)
            ctx_rtstA.__exit__(None, None, None)
            ctx_attnA.__exit__(None, None, None)
            ctx_psV.__exit__(None, None, None)
            ctx_psK.__exit__(None, None, None)
            ctx_mkin.__exit__(None, None, None)
            ctx_kvw.__exit__(None, None, None)

            # pairs 1-3: 3 psQK bufs
            with tc.tile_pool(name='attnB', bufs=6) as apoolB, \
                 tc.tile_pool(name='rtstB', bufs=6) as rtstB, \
                 tc.tile_pool(name='psQKB', bufs=3, space='PSUM') as psQKB:
                for sc in range(1, N_SC):
                    chunk_pair(sc, apoolB, rtstB, psQKB)
                    preload(sc)
                    # AG(sc-2) completed during chunk sc-1, so this gather
                    # and reciprocal never block the queues mid-attention
                    if sc >= 2:
                        gather_ci(sc - 2)
                        recip_ci(sc - 2)
            ctx_psAV.__exit__(None, None, None)
            ctx_proj.__exit__(None, None, None)

            # ---------------- epilogue (own s-slice) ----------------
            with tc.tile_pool(name='ep', bufs=1) as ep, \
                 tc.tile_pool(name='ept', bufs=3) as ept:
                rtn = ep.tile([128, 8 * 512], FP8, tag='rtn')
                rtnv = rtn[:].rearrange('p (kc s) -> p kc s', kc=8)
                wo8v = wo_bf[:].rearrange('p (kc c) -> p kc c', kc=8)
                wg18v = wg1_bf[:].rearrange('p (kc c) -> p kc c', kc=16)
                qs8v = qs_bf[:].rearrange('p (kc s) -> p kc s', kc=8)

                def normalize_ci(ci, psN):
                    for p in range(2):
                        kc = 2 * ci + p
                        bcp = psN.tile([128, 512], F32, tag='bcp',
                                       name=f'bcp{ci}{p}')
                        nc.tensor.matmul(bcp[:],
                                         sel4_sb[:, 128 * p:128 * (p + 1)],
                                         rdr_all[:, 512 * ci:512 * (ci + 1)],
                                         start=True, stop=True)
                        nc.vector.tensor_tensor(
                            rtnv[:, 2 * ci + p],
                            rawk[:, 512 * kc:512 * (kc + 1)], bcp[:], ALU.mult)

                # chunks 2 (AG done during chunk 3) and 3 (own slice, no AG)
                gather_ci(2)
                recip_ci(2)
                gather_ci(3)
                recip_ci(3)
                ctx_psN = tc.tile_pool(name='psN', bufs=2, space='PSUM')
                psN = ctx_psN.__enter__()
                for ci in range(3):
                    normalize_ci(ci, psN)

                gq_sb = ep.tile([128, 8 * 512], F32, tag='gq_sb')
                # Wg1 @ q for both hidden halves — no AG dependency
                for half in range(2):
                    with tc.tile_pool(name=f'psGQ{half}', bufs=1,
                                      space='PSUM') as psGQ:
                        pgq = [psGQ.tile([128, 512], F32, tag=f'pgq{i}',
                                         name=f'pgq{half}_{i}') for i in range(4)]
                        for cj in range(4):
                            for i in range(4):
                                dt = 4 * half + i
                                nc.tensor.matmul(
                                    pgq[i][:],
                                    wg18v[:, 2 * cj:2 * cj + 2,
                                          128 * dt:128 * (dt + 1)],
                                    qs8v[:, 2 * cj:2 * cj + 2, :],
                                    start=(cj == 0), stop=(cj == 3),
                                    perf_mode=PM.DoubleRow)
                        for i in range(4):
                            dt = 4 * half + i
                            nc.vector.tensor_copy(
                                gq_sb[:, 512 * dt:512 * (dt + 1)], pgq[i][:])

                # normalize the own-slice chunk (recip ran during Wg1@q)
                normalize_ci(3, psN)
                ctx_psN.__exit__(None, None, None)

                # Wo projection interleaved with Wg1@o half 0
                oT = ep.tile([128, 8 * SSL], BF16, tag='oT')
                oT8 = ep.tile([128, 8 * SSL], FP8, tag='oT8')
                oT8v = oT8[:].rearrange('p (dt s) -> p dt s', dt=8)
                sl = ep.tile([128, 8 * 512], F32R, tag='sl')

                def silu_block(pg, dt):
                    hg = ept.tile([128, 512], F32, tag='hg', name=f'hg{dt}')
                    nc.vector.tensor_tensor(
                        hg[:], pg[:], gq_sb[:, 512 * dt:512 * (dt + 1)], ALU.add)
                    sg = ept.tile([128, 512], F32, tag='sg', name=f'sg{dt}')
                    nc.scalar.activation(sg[:], hg[:], AF.Sigmoid,
                                         bias=bg1_sb[:, dt:dt + 1])
                    gg = ept.tile([128, 512], F32, tag='gg', name=f'gg{dt}')
                    nc.vector.tensor_scalar_add(gg[:], hg[:], bg1_sb[:, dt:dt + 1])
                    nc.vector.tensor_tensor(
                        sl[:, 512 * dt:512 * (dt + 1)], gg[:], sg[:], ALU.mult)

                def wg1o_step(pgo, half, cpair, start, stop):
                    for i in range(4):
                        nc.tensor.matmul(
                            pgo[i][:],
                            wg18v[:, 8 + 2 * cpair:8 + 2 * cpair + 2,
                                  512 * half + 128 * i:512 * half + 128 * (i + 1)],
                            oT8v[:, 2 * cpair:2 * cpair + 2, :],
                            start=start, stop=stop,
                            perf_mode=PM.DoubleRow)

                ctx_psG0 = tc.tile_pool(name='psG0', bufs=1, space='PSUM')
                psG0 = ctx_psG0.__enter__()
                pgo0 = [psG0.tile([128, 512], F32, tag=f'pgo0_{i}',
                                  name=f'pgo0_{i}') for i in range(4)]
                ctx_psWo = tc.tile_pool(name='psWo', bufs=2, space='PSUM')
                psWo = ctx_psWo.__enter__()
                for dt in range(8):
                    po = psWo.tile([128, 512], F32, tag='po')
                    for cj in range(4):
                        nc.tensor.matmul(
                            po[:], wo8v[:, 2 * cj:2 * cj + 2,
                                        128 * dt:128 * (dt + 1)],
                            rtnv[:, 2 * cj:2 * cj + 2, :],
                            start=(cj == 0), stop=(cj == 3),
                            perf_mode=PM.DoubleRow)
                    nc.vector.tensor_scalar_add(
                        oT[:, SSL * dt:SSL * (dt + 1)], po[:], bo2_sb[:, dt:dt + 1])
                    nc.scalar.activation(oT8v[:, dt],
                                         po[:], AF.Copy, scale=1.0 / 16.0)
                    # Wg1 @ o (half 0) lags so PE never waits on the oT8
                    # ACT copies; pair c ready after dt=2c+1
                    if dt >= 3 and dt % 2 == 1:
                        wg1o_step(pgo0, 0, (dt - 3) // 2,
                                  start=(dt == 3), stop=False)
                ctx_psWo.__exit__(None, None, None)
                wg1o_step(pgo0, 0, 2, start=False, stop=False)
                wg1o_step(pgo0, 0, 3, start=False, stop=True)
                for i in range(4):
                    silu_block(pgo0[i], i)
                ctx_psG0.__exit__(None, None, None)

                with tc.tile_pool(name='psG1', bufs=1, space='PSUM') as psG1:
                    pgo1 = [psG1.tile([128, 512], F32, tag=f'pgo1_{i}',
                                      name=f'pgo1_{i}') for i in range(4)]
                    for cpair in range(4):
                        wg1o_step(pgo1, 1, cpair,
                                  start=(cpair == 0), stop=(cpair == 3))
                    for i in range(4):
                        silu_block(pgo1[i], 4 + i)

                # gate scalar: sigmoid(Wg2 @ sl + bg2), broadcast to 128 rows
                with tc.tile_pool(name='psT', bufs=1, space='PSUM') as psT:
                    pgt = psT.tile([2, 512], F32, tag='pgt')
                    for kc in range(8):
                        nc.tensor.matmul(pgt[:], wg2_sb[:, 2 * kc:2 * (kc + 1)],
                                         sl[:, 512 * kc:512 * (kc + 1)],
                                         start=(kc == 0), stop=(kc == 7))
                    gate = ep.tile([2, 512], F32R, tag='gate')
                    nc.scalar.activation(gate[:], pgt[:], AF.Sigmoid, bias=bg2_sb[:])
                    gb = psT.tile([128, 512], F32, tag='gb')
                    nc.tensor.matmul(gb[:], bc0_sb[:], gate[:], start=True, stop=True)
                    gbs = ep.tile([128, 512], F32, tag='gbs')
                    nc.vector.tensor_copy(gbs[:], gb[:])

                    # out = q + gate * o
                    for dt in range(8):
                        go = ept.tile([128, 512], F32, tag='go')
                        nc.vector.tensor_tensor(
                            go[:], gbs[:], oT[:, SSL * dt:SSL * (dt + 1)], ALU.mult)
                        fo = ept.tile([128, 512], F32, tag='fo')
                        nc.vector.tensor_tensor(
                            fo[:], go[:], qs_sb[:, SSL * dt:SSL * (dt + 1)], ALU.add)
                        nc.sync.dma_start(out=out_t[128 * dt:128 * (dt + 1), :],
                                          in_=fo[:])

    nc.compile()
    return nc


def _shard(inputs):
    import ml_dtypes
    _bf16 = ml_dtypes.bfloat16
    _fp8 = ml_dtypes.float8_e4m3fn
    q = np.asarray(inputs['query'], np.float32)
    mk = np.asarray(inputs['memory_keys'], np.float32)
    mv = np.asarray(inputs['memory_values'], np.float32)
    Wq = np.asarray(inputs['Wq'], np.float32); bq = np.asarray(inputs['bq'], np.float32)
    Wk = np.asarray(inputs['Wk'], np.float32); bk = np.asarray(inputs['bk'], np.float32)
    Wv = np.asarray(inputs['Wv'], np.float32); bv = np.asarray(inputs['bv'], np.float32)
    Wo = np.asarray(inputs['Wo'], np.float32); bo = np.asarray(inputs['bo'], np.float32)
    Wg1 = np.asarray(inputs['Wg1'], np.float32); bg1 = np.asarray(inputs['bg1'], np.float32)
    Wg2 = np.asarray(inputs['Wg2'], np.float32); bg2 = np.asarray(inputs['bg2'], np.float32)

    scale = Dh ** -0.5
    bo2 = bo + Wo @ bv
    # the o-half of Wg1 sees oT8 = o - bo2, so fold Wg1o @ bo2 into bg1
    bg1_eff = bg1 + Wg1[:, DM:] @ bo2
    bc0 = np.zeros((2, 128), np.float32)
    bc0[0, :] = 1.0
    wg2T = np.zeros((DM, 2), np.float32)
    wg2T[:, 0] = Wg2[0]
    bg2v = np.zeros((2, 1), np.float32)
    bg2v[:, 0] = bg2[0]
    # sel4[2p + j//64, 128p + j] = 1/16 — picks denominator-recip row
    # 2p+h and pre-scales rtn into fp8 range (undone by the 16x in Wo)
    _sel4 = np.zeros((128, 256), np.float32)
    for _p in range(2):
        for _j in range(128):
            _sel4[2 * _p + _j // 64, 128 * _p + _j] = 1.0 / 16.0

    qT_b = [np.ascontiguousarray(q[b].T) for b in range(B)]
    mkT_b = [np.ascontiguousarray(mk[b].T) for b in range(B)]
    mvT_b = [np.ascontiguousarray(mv[b].T) for b in range(B)]
    WoT = np.ascontiguousarray(Wo.T)     # [1024 in, 1024 out]
    Wg1T = np.ascontiguousarray(Wg1.T)   # [2048 in, 1024 out]

    in_maps = []
    for c in range(NC):
        b, g = c // GS, c % GS
        hs = slice(64 * 4 * g, 64 * (4 * g + 4))  # rows of W for this core's 4 heads
        # s-rotation: compile chunk i processes logical slice (g+1+i)%4
        lsl = [(g + 1 + i) % 4 for i in range(4)]
        qT_c = np.concatenate([qT_b[b][:, 512 * l:512 * (l + 1)] for l in lsl],
                              axis=1)
        # arrival chunk ci comes from group-rank r_i = (g-1-ci)%4; its pair-p
        # block maps to Wo/Wg1 input-channel block 2*r_i + p
        ch = [2 * ((g - 1 - ci) % 4) + p for ci in range(4) for p in range(2)]
        woT_c = np.concatenate([WoT[128 * cb:128 * (cb + 1), :] for cb in ch])
        wg1T_c = np.concatenate(
            [Wg1T[0:1024, :]]
            + [Wg1T[1024 + 128 * cb:1024 + 128 * (cb + 1), :] for cb in ch])
        # gather row of (ci, p, head h, dim d):
        #   ci<3: 264*(4b + r_i) + 132p + 66h + d   in rt_og[ci]
        #   ci=3: 264*3        + 132p + 66h + d     in rt_in (own slice)
        def _base(ci, p):
            if ci < 3:
                return 264 * (4 * b + (g - 1 - ci) % 4) + 132 * p
            return 132 * p
        _gidx = np.asarray(
            [[_base(kc // 2, kc % 2) + 66 * (j // 64) + (j % 64)
              for j in range(128)] for kc in range(8)], np.int32)
        # denominator rows: j = 2p + h (4 valid); junk rows point at a
        # denominator row too (never zero, keeps 1/x finite)
        _didx = np.asarray(
            [[_base(ci, (j // 2) % 2) + 66 * (j % 2) + 64 if j < 4
              else _base(ci, 0) + 64 for j in range(128)]
             for ci in range(4)], np.int32)
        qs_c = np.ascontiguousarray(q[b].T[:, SSL * g:SSL * (g + 1)])

        in_maps.append({
            'qT': np.ascontiguousarray(qT_c),
            'mkT': mkT_b[b],
            'mvT': mvT_b[b],
            'wqT': np.ascontiguousarray((Wq[hs] * scale).T),
            'wkT': np.ascontiguousarray(Wk[hs].T),
            'wvT': np.ascontiguousarray(Wv[hs].T),
            'woT': np.ascontiguousarray((woT_c * 16.0).astype(_fp8)),
            'wg1T': np.ascontiguousarray((wg1T_c * 16.0).astype(_fp8)),
            'wg2T': wg2T,
            'qsT': qs_c,
            'qsbT': np.ascontiguousarray((qs_c / 16.0).astype(_fp8)),
            'vones': np.ascontiguousarray(np.tile([1.0, 0.0], 4)[None, :].repeat(128, 0).astype(_fp8)),
            'bc0': bc0,
            'bqv': np.ascontiguousarray((bq[hs] * scale).reshape(2, 128)),
            'bkv': np.ascontiguousarray(bk[hs].reshape(2, 128)),
            'bo2v': np.ascontiguousarray(bo2.reshape(8, 128)),
            'bg1v': np.ascontiguousarray(bg1_eff.reshape(8, 128)),
            'bg2v': bg2v,
            'gidx': _gidx,
            'didx': _didx,
            'sel4': np.ascontiguousarray(_sel4.astype(_bf16)),
        })
    return in_maps


def _run(inputs, trace=False):
    global _PROG
    from concourse.bass_utils import run_bass_kernel_spmd
    if _PROG is None:
        _PROG = _build_program()
    in_maps = _shard(inputs)
    res = run_bass_kernel_spmd(_PROG, in_maps, list(range(NC)), trace=trace)
    out = np.empty((B, S, DM), np.float32)
    for c in range(NC):
        b, g = c // GS, c % GS
        out[b, SSL * g:SSL * (g + 1), :] = res.results[c]['out_t'].T
    return out, res


def kernel(**inputs) -> np.ndarray:
    out, _ = _run(inputs, trace=False)
    return out
